# revision 1
# baseline (speedup 1.0000x reference)
"""Trainium2 Bass kernel for nn_AttentionLayer (sparse_attention).

Reference computation (per batch b):
    q = wq @ x + bq          [8, N]     (1x1 conv, d=8, N=H*W=4096)
    k = wk @ x + bk          [8, N]
    v = wv @ x + bv          [64, N]
    energy = q^T k           [N, N]
    attn = softmax(energy, axis=-1)
    out = gamma * (v @ attn^T) + x

Sharding: data-parallel over batch; 8 batches -> 8 NeuronCores, one batch
element per core.  Weights replicated.  No collectives.

Per-core layout strategy (avoids all large transposes; ~170us/core):
  - x_aug [65, 4096] sbuf, row 64 = ones (bias via augmented matmul); kept
    in f32 for the final residual add and in bf16 for matmul operands
    (fp32 matmuls lower to 2 PE passes at ~3x the cost - avoid).
  - q (pre-scaled by 1/16) and k are replicated 16x over the 128 partition
    rows, so the d=8 energy contraction becomes a K=128 matmul: a K=8
    matmul lights up 8/128 PE rows and the HAM clock gate then holds the
    PE at 1.2 GHz; the replicated form runs at 2.4 GHz and is exact.
  - vT [N, 64] computed directly transposed (lhsT = x chunk), augmented
    with a 65th column of ones so the attention matmul also produces the
    softmax denominator s[i] (row 64 of the psum output).
  - energy computed transposed, two i-chunks per psum tile: eT[j, i] ->
    [128(j), 1024(i)] f32 (one exp instruction per tile amortizes the
    per-op overhead); out_psum[65, i] accumulated over all 32 j-blocks.
    Softmax max-subtraction is unnecessary (|e| < ~4.5).
  - exp split across engines so the PE never stalls: ScalarE true exp for
    22 of 32 j-blocks, VectorE Schraudolph exp2 in the bf16 bit domain
    (one tensor_scalar: bitcast(int16(A*e + B))) for the rest, scheduled
    in the jb>=16 window clear of the normalize chains.
  - software-pipelined emission (energy PIPE=3 blocks ahead of the
    out-matmul) + psum pools sized so the in-order PE queue never waits:
    e [128,1024]x3 (6 banks) + out [65,512]x2 + borrowed slots.
  - normalize: 1/s by magic-constant seed + one Newton step (cheap DVE
    ops), broadcast over partitions via a K=1 matmul with -1 weights,
    emitted deferred/two-stage into the next pair so nothing the PE waits
    on queues behind it.

Accuracy: the attention term is ~1% of the output magnitude (the residual
dominates), so bf16 operands + 3% Schraudolph exp + 0.3% reciprocal cost
~3e-4 final relative error (tolerance 2e-2).
"""

import os
import sys

import numpy as np

sys.path.insert(0, "/opt/trn_rl_repo")

B, C, HH, WW = 8, 64, 64, 64
N = HH * WW  # 4096
D = 8  # qk channels
IC = 512  # i-chunk (queries per psum accumulation)
N_IC = N // IC  # 8
JB = 128  # j-block (keys per energy tile)
N_JB = N // JB  # 32

_CACHE = {}


def _build_program():
    import concourse.bass as bass
    import concourse.tile as tile
    from concourse import bacc, mybir
    from concourse.masks import make_identity

    f32 = mybir.dt.float32
    bf16 = mybir.dt.bfloat16

    nc = bacc.Bacc(
        "TRN2", target_bir_lowering=False, debug=False, enable_asserts=False
    )

    x_d = nc.dram_tensor("x", [C, N], f32, kind="ExternalInput").ap()
    wq_d = nc.dram_tensor("wq", [D, C], f32, kind="ExternalInput").ap()
    bq_d = nc.dram_tensor("bq", [D], f32, kind="ExternalInput").ap()
    wk_d = nc.dram_tensor("wk", [D, C], f32, kind="ExternalInput").ap()
    bk_d = nc.dram_tensor("bk", [D], f32, kind="ExternalInput").ap()
    wv_d = nc.dram_tensor("wv", [C, C], f32, kind="ExternalInput").ap()
    bv_d = nc.dram_tensor("bv", [C], f32, kind="ExternalInput").ap()
    gamma_d = nc.dram_tensor("gamma", [1], f32, kind="ExternalInput").ap()
    y_d = nc.dram_tensor("y", [C, N], f32, kind="ExternalOutput").ap()

    EXP = mybir.ActivationFunctionType.Exp

    with tile.TileContext(nc) as tc:
        from contextlib import ExitStack

        with ExitStack() as ctx:
            consts = ctx.enter_context(tc.tile_pool(name="consts", bufs=1))
            bigs = ctx.enter_context(tc.tile_pool(name="bigs", bufs=1))
            work = ctx.enter_context(tc.tile_pool(name="work", bufs=8))
            ypool = ctx.enter_context(tc.tile_pool(name="ypool", bufs=4))
            small = ctx.enter_context(tc.tile_pool(name="small", bufs=4))


            # ---------------- constants / weights prep ----------------
            ident = consts.tile([C, C], f32)
            make_identity(nc, ident)

            mones = consts.tile([65, C], bf16)
            nc.vector.memset(mones, -1.0)

            # warm the Exp activation table early so the ~2.7us table load
            # overlaps the prep phase
            warm = consts.tile([1, 8], f32)
            nc.scalar.activation(warm, ident[0:1, 0:8], EXP)

            gcol = consts.tile([65, 1], f32)
            nc.sync.dma_start(out=gcol, in_=gamma_d.to_broadcast([65, 1]))

            x_aug = bigs.tile([65, N], f32)
            x_bf = bigs.tile([65, N], bf16)
            nc.vector.memset(x_bf[C : C + 1, :], 1.0)

            # raw weights first (tiny DMAs; must not queue behind the
            # 1 MB x transfer)
            wq_sb = consts.tile([D, C], f32)
            wk_sb = consts.tile([D, C], f32)
            wv_sb = consts.tile([C, C], f32)
            nc.sync.dma_start(out=wq_sb, in_=wq_d)
            nc.sync.dma_start(out=wk_sb, in_=wk_d)
            nc.sync.dma_start(out=wv_sb, in_=wv_d)

            # fold gamma into wv (and bv below): out = gamma*(v@attnT) + x
            nc.vector.tensor_scalar_mul(wv_sb, wv_sb, gcol[0:C])
            # q is replicated 16x over the 128 contraction partitions with a
            # 1/16 scale folded into wq/bq, so the K=128 energy matmul
            # computes exactly q.k (16 * q.k/16).  K=8 matmuls light up only
            # 8 of 128 PE rows and keep the HAM clock gate at 1.2 GHz; the
            # replicated K=128 form runs the array at full rate (2.4 GHz).
            nc.vector.tensor_scalar_mul(wq_sb, wq_sb, 1.0 / 16.0)

            # wqT_rep / wkT_rep: [65, 64] bf16; cols [0:8] and [32:40] hold
            # the transposed weights, row 64 holds the bias (matches x ones
            # row).  Other columns zero.
            wqT = consts.tile([65, 2 * C], bf16)
            wkT = consts.tile([65, 2 * C], bf16)

            # prep-phase psum pool: scoped so its banks are
            # released before the main-loop pools are in use
            with tc.tile_pool(
                name="psum_x", bufs=4, space="PSUM"
            ) as psum_x:
                # bias rows live on partition 64: stage there via DMA,
                # scale bq by 1/16 to match the replicated-q trick
                bst64 = consts.tile([65, 2 * D + C], f32)
                nc.sync.dma_start(out=bst64[C : C + 1, 0:D], in_=bq_d[None, :])
                nc.sync.dma_start(
                    out=bst64[C : C + 1, D : 2 * D], in_=bk_d[None, :]
                )
                nc.sync.dma_start(out=bst64[C : C + 1, 2 * D :], in_=bv_d[None, :])
                nc.vector.tensor_scalar_mul(
                    bst64[C : C + 1, 0:D], bst64[C : C + 1, 0:D], 1.0 / 16.0
                )

                wqT8 = consts.tile([65, D], bf16)
                pt = psum_x.tile([C, D], f32, tag="px")
                nc.tensor.transpose(pt, wq_sb, ident[0:D, 0:D])
                nc.vector.tensor_copy(out=wqT8[0:C, :], in_=pt)
                nc.vector.tensor_copy(
                    out=wqT8[C : C + 1, :], in_=bst64[C : C + 1, 0:D]
                )

                wkT8 = consts.tile([65, D], bf16)
                pt2 = psum_x.tile([C, D], f32, tag="px")
                nc.tensor.transpose(pt2, wk_sb, ident[0:D, 0:D])
                nc.vector.tensor_copy(out=wkT8[0:C, :], in_=pt2)
                nc.vector.tensor_copy(
                    out=wkT8[C : C + 1, :], in_=bst64[C : C + 1, D : 2 * D]
                )

                for w_dst, w_src in ((wqT, wqT8), (wkT, wkT8)):
                    sap = w_src[:]
                    rep = bass.AP(
                        tensor=sap.tensor,
                        offset=sap.offset,
                        ap=[sap.ap[0], [0, 16], sap.ap[1]],
                    )
                    nc.vector.tensor_copy(
                        out=w_dst.rearrange("p (g d) -> p g d", g=16),
                        in_=rep,
                    )

                # wvT_aug [65, 64] bf16: rows 0:64 = (gamma*wv)^T, row 64 = gamma*bv
                wvT = consts.tile([65, C], bf16)
                nc.vector.tensor_scalar_mul(
                    bst64[C : C + 1, 2 * D :], bst64[C : C + 1, 2 * D :],
                    gcol[C : C + 1],
                )
                nc.vector.tensor_copy(
                    out=wvT[C : C + 1, :], in_=bst64[C : C + 1, 2 * D :]
                )
                pt3 = psum_x.tile([C, C], f32, tag="px")
                nc.tensor.transpose(pt3, wv_sb, ident)
                nc.vector.tensor_copy(out=wvT[0:C, :], in_=pt3)

                # ---------------- projections ----------------
                # x_aug: [65, N] f32, row 64 = ones; x_bf: bf16 copy for
                # matmuls.  DMA'd and cast per 512-chunk, emitted after the
                # weight prep so those DVE ops aren't queued behind 8 casts.
                for ic in range(N_IC):
                    sl = slice(ic * IC, (ic + 1) * IC)
                    nc.gpsimd.dma_start(out=x_aug[0:C, sl], in_=x_d[:, sl])
                    nc.vector.tensor_copy(
                        out=x_bf[0:C, sl], in_=x_aug[0:C, sl]
                    )

                # qk_sb [128, 2, N] bf16: q (scaled 1/16) and k, each
                # replicated 16x over partitions; plane 0 = q, plane 1 = k.
                # vT_aug [128, 32, 65] bf16: vT[p, jc, 0:64] = v^T[jc*128+p],
                # vT[:, :, 64] = 1.0 (produces softmax denominator s).
                # All pipelined per 512-column chunk behind the x DMA.
                qk_sb = bigs.tile([2 * C, 2, N], bf16)
                vT = bigs.tile([JB, N_JB, C + 1], bf16)
                nc.vector.memset(vT[:, :, C : C + 1], 1.0)
                for ic in range(N_IC):
                    sl = slice(ic * IC, (ic + 1) * IC)
                    pq = psum_x.tile([2 * C, IC], f32, tag="px")
                    nc.tensor.matmul(pq, wqT, x_bf[:, sl], start=True, stop=True)
                    nc.vector.tensor_copy(out=qk_sb[:, 0, sl], in_=pq)
                    pk = psum_x.tile([2 * C, IC], f32, tag="px")
                    nc.tensor.matmul(pk, wkT, x_bf[:, sl], start=True, stop=True)
                    nc.vector.tensor_copy(out=qk_sb[:, 1, sl], in_=pk)
                    pv = psum_x.tile([JB, 4 * C], f32, tag="px")
                    for j4 in range(4):
                        jc = ic * 4 + j4
                        nc.tensor.matmul(
                            pv[:, j4 * C : (j4 + 1) * C],
                            x_bf[:, jc * JB : (jc + 1) * JB],
                            wvT,
                            start=True,
                            stop=True,
                        )
                    nc.vector.tensor_copy(
                        out=vT[:, ic * 4 : (ic + 1) * 4, 0:C],
                        in_=pv.rearrange("p (a b) -> p a b", a=4),
                    )

            psum_e = ctx.enter_context(
                tc.tile_pool(name="psum_e", bufs=3, space="PSUM")
            )
            psum_o = ctx.enter_context(
                tc.tile_pool(name="psum_o", bufs=2, space="PSUM")
            )

            # ---------------- main attention loop ----------------
            # Software-pipelined emission: the PE instruction queue is
            # in-order, so the out-matmul for block jb (which waits on the
            # exp) must not sit in front of the energy matmul for jb+1..jb+P.
            # Emit energy P blocks ahead of the out-matmul.
            #
            # exp is split between ScalarE (true exp, ~684ns/block) and
            # VectorE (Schraudolph exp2 in the bf16 bit domain, one
            # tensor_scalar op) so that combined exp throughput beats the
            # PE's ~450ns/iter and the PE never stalls (stalls drain the MM
            # pipeline and drop the PE clock to 1.2 GHz).
            #
            # Schraudolph: bf16 = [sign | 8-bit exp | 7-bit mant] so
            # bitcast(int16(A*e + B)) ~= exp(e) with A = 128*log2(e),
            # B = 128*127 - 5.5 (max rel err ~3.3%; the attention term is
            # ~1% of the output so this costs ~1e-3 final relative error).
            # DVE exp blocks live in the jb>=16 window so the deferred
            # normalize chains (emitted at jb 2/8) never sit in front of a
            # Schraudolph exp the PE is about to need
            DVE_SET = {13, 16, 18, 19, 21, 22, 24, 25, 27, 28, 30, 31}
            SCH_A = float(128.0 / np.log(2.0))
            SCH_B = float(128.0 * 127.0 - 5.5)
            i16 = mybir.dt.int16
            i32 = mybir.dt.int32
            PIPE = 3
            # Two i-chunks are processed as one stream: both energy blocks
            # land in one [128, 1024] psum tile (2 banks) so a single exp
            # instruction covers FD=1024, amortizing the per-instruction
            # overhead (ACT 573ns, DVE 596ns per 512-block equivalent).
            # Combined exp throughput then beats the PE pace (~442ns/blk)
            # and the PE runs stall-free.
            # Deferred, two-stage normalize.  Stage 1 (reciprocal of s +
            # bf16 cast, both VectorE) is emitted early in the next pair so
            # it sits behind nothing the PE needs.  Stage 2 (broadcast
            # matmul, multiply, residual add, store) is emitted LATE in the
            # next pair: the broadcast matmul borrows an e-pool psum slot,
            # and pool slots are granted in request order - requesting early
            # while its input is still being computed would wedge the whole
            # energy pipeline behind it.
            norm_s1 = []
            norm_s2 = []

            def emit_norm_s1(yu, sl):
                # 1/s via magic-constant seed + one Newton step using only
                # cheap 1-pass DVE ops (the iterative RECIPROCAL is 8
                # cycles/element, ~3.3us per row).  r0 = bitcast(K - s_int)
                # has ~4% error; (s*r0 - 2)*r0 = -1/s to ~0.3%.  The minus
                # sign is absorbed by broadcasting with -1 weights.
                r0i = small.tile([C + 1, IC], i32, tag="r0")
                nc.vector.tensor_scalar(
                    r0i[C : C + 1, :],
                    yu[C : C + 1, :].bitcast(i32),
                    -1.0,
                    float(0x7EF33333),
                    op0=mybir.AluOpType.mult,
                    op1=mybir.AluOpType.add,
                )
                r0 = r0i.bitcast(f32)
                t_sb = small.tile([C + 1, IC], f32, tag="r")
                nc.vector.tensor_mul(
                    t_sb[C : C + 1, :], yu[C : C + 1, :], r0[C : C + 1, :]
                )
                r_bf = small.tile([C + 1, IC], bf16, tag="rbf")
                nc.vector.scalar_tensor_tensor(
                    out=r_bf[C : C + 1, :],
                    in0=t_sb[C : C + 1, :],
                    scalar=2.0,
                    in1=r0[C : C + 1, :],
                    op0=mybir.AluOpType.subtract,
                    op1=mybir.AluOpType.mult,
                )
                norm_s2.append(
                    (lambda yu=yu, sl=sl, r_bf=r_bf: emit_norm_s2(yu, sl, r_bf))
                )

            def emit_norm_s2(yu, sl, r_bf):
                rb_ps = psum_e.tile([C, IC], f32, tag="e_ps")
                nc.tensor.matmul(
                    rb_ps,
                    mones[C : C + 1, 0:C],
                    r_bf[C : C + 1, :],
                    start=True,
                    stop=True,
                )
                y_sb = ypool.tile([C, IC], f32)
                nc.vector.tensor_mul(y_sb, yu[0:C, :], rb_ps)
                nc.vector.tensor_add(y_sb, y_sb, x_aug[0:C, sl])
                nc.sync.dma_start(out=y_d[:, sl], in_=y_sb)

            for icp in range(N_IC // 2):
                slA = slice((2 * icp) * IC, (2 * icp + 1) * IC)
                slB = slice((2 * icp + 1) * IC, (2 * icp + 2) * IC)
                out_A = psum_o.tile([C + 1, IC], f32, tag="op")
                out_B = psum_o.tile([C + 1, IC], f32, tag="op")
                a_tiles = {}
                for jb in range(N_JB + PIPE):
                    # emit deferred normalize chains from the previous pair
                    # mid-loop, so their DVE ops queue BEHIND a few of this
                    # pair's Schraudolph exps instead of starving the PE at
                    # the pair boundary
                    if norm_s1 and jb == 2:
                        norm_s1.pop(0)()
                    if norm_s1 and jb == 8:
                        norm_s1.pop(0)()
                    if norm_s2 and jb == 26:
                        norm_s2.pop(0)()
                    if norm_s2 and jb == 29:
                        norm_s2.pop(0)()
                    if jb < N_JB:
                        e_ps = psum_e.tile([JB, 2 * IC], f32)
                        ksl = qk_sb[:, 1, jb * JB : (jb + 1) * JB]
                        # eT[j, i] = sum_d k[d, j] * q[d, i], both chunks;
                        # contraction over 128 partitions = 16 replicas of
                        # the d=8 dot product (q pre-scaled by 1/16)
                        nc.tensor.matmul(
                            e_ps[:, 0:IC], ksl, qk_sb[:, 0, slA],
                            start=True, stop=True,
                        )
                        nc.tensor.matmul(
                            e_ps[:, IC : 2 * IC], ksl, qk_sb[:, 0, slB],
                            start=True, stop=True,
                        )
                        if jb in DVE_SET:  # VectorE handles these pairs
                            aTi = work.tile([JB, 2 * IC], i16, tag="aT")
                            nc.vector.tensor_scalar(
                                aTi,
                                e_ps,
                                SCH_A,
                                SCH_B,
                                op0=mybir.AluOpType.mult,
                                op1=mybir.AluOpType.add,
                            )
                            aT = aTi.bitcast(bf16)
                        else:
                            aT = work.tile([JB, 2 * IC], bf16, tag="aT")
                            nc.scalar.activation(aT, e_ps, EXP)
                        a_tiles[jb] = aT
                    jo = jb - PIPE
                    if jo >= 0:
                        # out_un[c, i] += sum_j vT[j, c] * aT[j, i]
                        # row 64 accumulates s[i] = sum_j aT[j, i]
                        aT = a_tiles.pop(jo)
                        nc.tensor.matmul(
                            out_A,
                            vT[:, jo, :],
                            aT[:, 0:IC],
                            start=(jo == 0),
                            stop=(jo == N_JB - 1),
                        )
                        nc.tensor.matmul(
                            out_B,
                            vT[:, jo, :],
                            aT[:, IC : 2 * IC],
                            start=(jo == 0),
                            stop=(jo == N_JB - 1),
                        )

                # normalize both chunks: evacuate out_un to SBUF on
                # ScalarE (frees the psum bank fast so the next pair's
                # out-matmuls are not blocked), then r = 1/s on VectorE,
                # broadcast r over 64 partitions via a K=1 matmul (borrows
                # an e-slot), multiply, add residual, store.
                # evacuate both accumulators to SBUF on ScalarE now (the
                # psum slots free after ~0.6us) and DEFER the DVE normalize
                # chain into the next pair's loop (jb 2 and 8)
                for out_ps, sl in ((out_A, slA), (out_B, slB)):
                    yu = small.tile([C + 1, IC], f32, tag="yu")
                    nc.scalar.copy(yu, out_ps)
                    norm_s1.append(
                        (lambda yu=yu, sl=sl: emit_norm_s1(yu, sl))
                    )

            # flush remaining deferred normalizes (last pair)
            while norm_s1:
                norm_s1.pop(0)()
            while norm_s2:
                norm_s2.pop(0)()

    nc.compile()
    return nc


def _get_program():
    if "nc" not in _CACHE:
        _CACHE["nc"] = _build_program()
    return _CACHE["nc"]


def kernel(**inputs) -> np.ndarray:
    import time

    nc = _get_program()
    from concourse.bass_utils import run_bass_kernel_spmd

    x = np.ascontiguousarray(np.asarray(inputs["x"], dtype=np.float32))
    shared = {
        k: np.ascontiguousarray(np.asarray(inputs[k], dtype=np.float32))
        for k in ("wq", "bq", "wk", "bk", "wv", "bv", "gamma")
    }
    in_maps = [
        {"x": x[b].reshape(C, N).copy(), **shared} for b in range(B)
    ]
    # the axon-tunneled device occasionally reports a transient
    # NRT_EXEC_UNIT_UNRECOVERABLE; a retry on a fresh execution succeeds
    last_err = None
    for attempt in range(4):
        try:
            res = run_bass_kernel_spmd(nc, in_maps, list(range(B)))
            break
        except Exception as e:  # noqa: BLE001
            last_err = e
            time.sleep(2.0 * (attempt + 1))
    else:
        raise last_err
    out = np.stack(
        [res.results[b]["y"].reshape(C, HH, WW) for b in range(B)], axis=0
    )
    return out.astype(np.float32)


if __name__ == "__main__":
    rng = np.random.default_rng(0)
    inputs = {
        "x": rng.standard_normal((B, C, HH, WW), dtype=np.float32),
        "wq": rng.standard_normal((D, C), dtype=np.float32) * 0.05,
        "bq": rng.standard_normal((D,), dtype=np.float32) * 0.05,
        "wk": rng.standard_normal((D, C), dtype=np.float32) * 0.05,
        "bk": rng.standard_normal((D,), dtype=np.float32) * 0.05,
        "wv": rng.standard_normal((C, C), dtype=np.float32) * 0.05,
        "bv": rng.standard_normal((C,), dtype=np.float32) * 0.05,
        "gamma": rng.standard_normal((1,), dtype=np.float32),
    }
    out = kernel(**inputs)
    print("out", out.shape, out.dtype, float(np.abs(out).max()))

